# revision 50
# baseline (speedup 1.0000x reference)
"""CRF negative-log-likelihood kernel for Trainium2 (8 NeuronCores, one launch).

Phase 1 (vocab-sharded): t2 = embedding @ fc_w in bf16. Host pre-transposes the
embedding shard; matmuls keep fc_w stationary (out 16 x 320 tiles); one XBAR
DMA-transpose repacks t2T (16, VSH) into the packed (128, 50*C) row layout.
The 8 shards are exchanged with an in-kernel AllGather (packed layout; the
host maps gather indices into it, so no device-side relayout is needed).

Phase 2 (batch-sharded, 8 rows/core, bf16): merged indirect-DMA gathers of t2
rows (8 calls, 4096 descriptors each, balanced chunk order 0,7,1,6,...), PE
block-transposes into class-on-partition layout (DMA engines stay free for
gather descriptors), and a segmented forward/backward scan (L=8, S=512) in
linear space whose two chains chase the gather chunks from both ends. The
host token permutation makes G segment-major so every scan read is contiguous.

Host (float64, vectorized): numerator start/end/transition terms, exact
segment-0 alpha, rank-1 junction chain across segments, exact ragged tails.
"""
import sys
sys.path.insert(0, "/opt/trn_rl_repo")
import numpy as np
import ml_dtypes
from contextlib import ExitStack

import concourse.bass as bass
import concourse.bacc as bacc_mod
import concourse.mybir as mybir
import concourse.tile as tile
from concourse.masks import make_identity
from concourse.bass_utils import run_bass_kernel_spmd

F32 = mybir.dt.float32
BF16 = mybir.dt.bfloat16
I32 = mybir.dt.int32
NPBF = ml_dtypes.bfloat16

V, E, C = 50257, 128, 16
B, T = 64, 4096
L, S = 8, 512
VPAD = 51200
VSH = VPAD // 8
BL = 8
NCHUNK = 8
CHW = T // NCHUNK
NCORES = 8
NTILE = VSH // 128          # 50

LAST_EXEC_NS = {}
_TRACE = False
_CACHE = {}


def build_merged_kernel():
    nc = bacc_mod.Bacc()
    embT_s = nc.dram_tensor("embT_s", [E, VSH], F32, kind="ExternalInput")
    fc_w = nc.dram_tensor("fc_w", [E, C], F32, kind="ExternalInput")
    x_t = nc.dram_tensor("x_t", [128, T // 128 * BL], I32, kind="ExternalInput")
    tags_f = nc.dram_tensor("tags_f", [BL, T], BF16, kind="ExternalInput")
    blockP = nc.dram_tensor("blockP", [128, 128], BF16, kind="ExternalInput")
    blockPT = nc.dram_tensor("blockPT", [128, 128], BF16, kind="ExternalInput")
    bcast8 = nc.dram_tensor("bcast8", [BL, 128], BF16, kind="ExternalInput")
    iota_rep = nc.dram_tensor("iota_rep", [128, CHW], BF16, kind="ExternalInput")

    r_out = nc.dram_tensor("r_out", [128, S], BF16, kind="ExternalOutput")
    d_out = nc.dram_tensor("d_out", [128, S], BF16, kind="ExternalOutput")
    num_out = nc.dram_tensor("num_out", [1, 16], F32, kind="ExternalOutput")
    t2_out = nc.dram_tensor("t2_out", [128, NTILE * C], BF16,
                            kind="ExternalOutput")

    # internal DRAM for the shard exchange; t2full flat = concat of the 8
    # packed (128, 50*C) shards, viewed as (VPAD, C) rows via the host-side
    # index mapping v -> k*6400 + (v%128)*50 + (v%6400)//128
    t2sh = nc.dram_tensor("t2sh", [128, NTILE * C], BF16)
    t2full = nc.dram_tensor("t2full", [VPAD, C], BF16, addr_space="Shared")

    NGRP = 10
    GW = VSH // NGRP            # 640
    MW = GW // 2                # 320 f32 fits a 2KB PSUM bank

    with ExitStack() as ctx:
        tc = ctx.enter_context(tile.TileContext(nc))
        singles = ctx.enter_context(tc.tile_pool(name="singles", bufs=1))
        big = ctx.enter_context(tc.tile_pool(name="big", bufs=1))
        scratch = ctx.enter_context(tc.tile_pool(name="scratch", bufs=3))
        psum = ctx.enter_context(tc.tile_pool(name="psum", bufs=2, space="PSUM"))
        psumT = ctx.enter_context(tc.tile_pool(name="psumT", bufs=1, space="PSUM"))
        psum2 = ctx.enter_context(tc.tile_pool(name="psum2", bufs=1, space="PSUM"))

        # ---- main-phase inputs issued first: they overlap phase 1 ----
        xt_sb = singles.tile([128, T // 128 * BL], I32)
        nc.sync.dma_start(out=xt_sb[:], in_=x_t[:])
        tagsf_sb = singles.tile([BL, T], BF16)
        nc.scalar.dma_start(out=tagsf_sb[:], in_=tags_f[:])
        bcast8_sb = singles.tile([BL, 128], BF16)
        nc.scalar.dma_start(out=bcast8_sb[:], in_=bcast8[:])
        iotar_sb = singles.tile([128, CHW], BF16)
        nc.scalar.dma_start(out=iotar_sb[:], in_=iota_rep[:])
        blockP_sb = singles.tile([128, 128], BF16)
        nc.scalar.dma_start(out=blockP_sb[:], in_=blockP[:])
        blockPT_sb = singles.tile([128, 128], BF16)
        nc.scalar.dma_start(out=blockPT_sb[:], in_=blockPT[:])

        # ---- phase 1: t2 shard ----
        fcw_f32 = singles.tile([E, C], F32)
        nc.scalar.dma_start(out=fcw_f32[:], in_=fc_w[:])
        fcw_bf = singles.tile([E, C], BF16)
        nc.vector.tensor_copy(fcw_bf[:], fcw_f32[:])

        EMBT = singles.tile([128, VSH], F32)
        EMBTb = singles.tile([128, VSH], BF16)
        T2T = singles.tile([C, VSH], BF16)
        for g in range(NGRP):
            eng = nc.sync if g % 2 == 0 else nc.scalar
            eng.dma_start(out=EMBT[:, g * GW:(g + 1) * GW],
                          in_=embT_s[:, g * GW:(g + 1) * GW])
            for h in range(2):
                m0 = g * GW + h * MW
                if h == 0:
                    nc.vector.tensor_copy(EMBTb[:, m0:m0 + MW],
                                          EMBT[:, m0:m0 + MW])
                else:
                    nc.scalar.copy(EMBTb[:, m0:m0 + MW], EMBT[:, m0:m0 + MW])
        for m in range(2 * NGRP):
            ps2 = psum.tile([C, MW], F32, tag="p2")
            nc.tensor.matmul(ps2[:], lhsT=fcw_bf[:],
                             rhs=EMBTb[:, m * MW:(m + 1) * MW],
                             start=True, stop=True)
            if m % 2 == 0:
                nc.vector.tensor_copy(T2T[:, m * MW:(m + 1) * MW], ps2[:])
            else:
                nc.scalar.copy(T2T[:, m * MW:(m + 1) * MW], ps2[:])

        # one XBAR repack: T2T (16, VSH) -> T2pk (128, 50*C); row v=i*128+p
        # lands at [p, i*C:(i+1)*C]
        T2pk = singles.tile([128, NTILE * C], BF16)
        T2pkap = T2pk[:]
        nc.sync.dma_start_transpose(
            out=bass.AP(tensor=T2pkap.tensor, offset=0,
                        ap=[[T2pkap.ap[0][0], 128], [C, NTILE], [1, C]]),
            in_=T2T[:])
        nc.scalar.dma_start(out=t2_out[:], in_=T2pk[:])
        nc.sync.dma_start(out=t2sh[:], in_=T2pk[:])

        # ---- exchange: AllGather the packed shards ----
        nc.gpsimd.collective_compute(
            "AllGather", mybir.AluOpType.bypass,
            replica_groups=[[0, 1, 2, 3, 4, 5, 6, 7]],
            ins=[t2sh[:].opt()], outs=[t2full[:].opt()])

        # ---- phase 2 ----
        TM = big.tile([128, T], BF16)
        G = big.tile([128, T], BF16)
        EXPG = big.tile([128, T], BF16)
        W = big.tile([128, T], BF16)
        num_sb = singles.tile([1, 16], F32)
        nc.vector.memset(num_sb[:], 0.0)
        ones_sb = singles.tile([128, 1], BF16)
        nc.vector.memset(ones_sb[:], 1.0)
        ident_bf = singles.tile([128, 128], BF16)
        make_identity(nc, ident_bf[:])
        r_sb = big.tile([128, S], BF16)
        nc.vector.memset(r_sb[:], 1.0)
        d_sb = big.tile([128, S], BF16)

        # Host permutes the token order so G is SEGMENT-MAJOR: G column
        # k*S + s = token s*L + k; chunk c holds scan step k=c's block, so
        # the forward scan chases the gather and (with the balanced order
        # below) the backward scan chases from the other end.
        GORDER = [0, 7, 1, 6, 2, 5, 3, 4]

        # numerator W build: only needs tags; runs during phase 1 / gather
        for c in GORDER:
            c0 = c * CHW
            psA = psum.tile([128, CHW], F32, tag="ps")
            nc.tensor.matmul(psA[:], lhsT=bcast8_sb[:],
                             rhs=tagsf_sb[:, c0:c0 + CHW], start=True, stop=True)
            nc.vector.tensor_tensor(out=W[:, c0:c0 + CHW], in0=psA[:],
                                    in1=iotar_sb[:], op=mybir.AluOpType.is_equal)

        # gather: one merged indirect DMA per chunk (4096 descriptors);
        # offsets enumerate (partition, col) C-order, 16 contiguous bf16 per
        # offset
        ncc = CHW // 16
        for c in GORDER:
            c0 = c * CHW
            nc.gpsimd.indirect_dma_start(
                out=TM[:, c0:c0 + CHW],
                out_offset=None,
                in_=t2full[:],
                in_offset=bass.IndirectOffsetOnAxis(
                    ap=xt_sb[:, c * ncc:(c + 1) * ncc], axis=0),
            )

        nbl = CHW // 128
        psN = psum2.tile([1, CHW], F32, tag="psN")

        def trans_exp(c):
            # PE block-transposes (DMA engines stay free for the gather);
            # copy back on DVE (2x bf16 mode)
            c0 = c * CHW
            psT = psumT.tile([128, CHW], BF16, tag="psT")
            for b in range(nbl):
                nc.tensor.transpose(psT[:, b * 128:(b + 1) * 128],
                                    TM[:, c0 + b * 128:c0 + (b + 1) * 128],
                                    ident_bf[:])
            nc.vector.tensor_copy(G[:, c0:c0 + CHW], psT[:])
            nc.scalar.activation(EXPG[:, c0:c0 + CHW], G[:, c0:c0 + CHW],
                                 mybir.ActivationFunctionType.Exp)

        ACC_ORDER = [0, 7, 1, 6, 2, 5, 3, 4]

        def numer(c):
            c0 = c * CHW
            scr2 = scratch.tile([128, CHW], BF16, tag=f"scr2_{c % 3}")
            nc.vector.tensor_mul(scr2[:], G[:, c0:c0 + CHW], W[:, c0:c0 + CHW])
            i = ACC_ORDER.index(c)
            nc.tensor.matmul(psN[:], lhsT=ones_sb[:], rhs=scr2[:],
                             start=(i == 0), stop=(i == NCHUNK - 1))

        def fw(k):
            psR = psum2.tile([128, S], F32, tag="psR")
            nc.tensor.matmul(psR[:], lhsT=blockP_sb[:], rhs=r_sb[:],
                             start=True, stop=True)
            nc.vector.tensor_mul(r_sb[:], psR[:], EXPG[:, k * S:(k + 1) * S])

        def bw(k):
            psD = psum2.tile([128, S], F32, tag="psD")
            nc.tensor.matmul(psD[:], lhsT=blockPT_sb[:], rhs=d_sb[:],
                             start=True, stop=True)
            nc.vector.tensor_mul(d_sb[:], psD[:], EXPG[:, k * S:(k + 1) * S])

        def dinit():
            nc.vector.tensor_copy(d_sb[:], EXPG[:, (L - 1) * S:L * S])

        # slot-ordered issue: each op runs as its chunk lands
        trans_exp(0); fw(0); numer(0)
        trans_exp(7); dinit(); numer(7)
        trans_exp(1); fw(1); numer(1)
        trans_exp(6); bw(6); numer(6)
        trans_exp(2); fw(2); numer(2)
        trans_exp(5); bw(5); numer(5)
        trans_exp(3); fw(3); numer(3)
        trans_exp(4); fw(4); numer(4)
        bw(4)
        fw(5)
        bw(3)
        fw(6)
        bw(2)
        fw(7)
        bw(1)
        bw(0)
        nc.vector.reduce_sum(out=num_sb[:, 0:1], in_=psN[:],
                             axis=mybir.AxisListType.X)

        nc.sync.dma_start(out=r_out[:], in_=r_sb[:])
        nc.sync.dma_start(out=num_out[:], in_=num_sb[:])
        nc.sync.dma_start(out=d_out[:], in_=d_sb[:])
    return nc


def _host_prep(embedding, fc_w, fc_b, trans, start):
    P_eff64 = np.exp(trans.astype(np.float64) + fc_b[None, :].astype(np.float64))
    trans_n = (trans + fc_b[None, :]).astype(np.float32)
    P_eff32 = P_eff64.astype(np.float32)

    eye8 = np.eye(BL, dtype=np.float32)
    return dict(
        P_eff=P_eff64,
        trans_n=trans_n.astype(np.float64),
        blockP=np.ascontiguousarray(np.kron(eye8, P_eff32)).astype(NPBF),
        blockPT=np.ascontiguousarray(np.kron(eye8, P_eff32.T.copy())).astype(NPBF),
        bcast8=np.ascontiguousarray(np.kron(eye8, np.ones((1, C), np.float32))).astype(NPBF),
        iota_rep=np.ascontiguousarray(
            np.tile(np.tile(np.arange(C, dtype=np.float32), BL)[:, None],
                    (1, CHW))).astype(NPBF),
    )


LAST_RESULTS = {}


def _run(nc, in_maps, label):
    res = run_bass_kernel_spmd(nc, in_maps, core_ids=list(range(NCORES)),
                               trace=_TRACE)
    if res.exec_time_ns is not None:
        LAST_EXEC_NS[label] = res.exec_time_ns
    LAST_RESULTS[label] = res
    return res.results


def kernel(x, tags, embedding, fc_w, fc_b, start_transitions, end_transitions,
           transitions):
    x = np.asarray(x, np.int32)
    tags = np.asarray(tags, np.int32)
    embedding = np.asarray(embedding, np.float32)
    fc_w = np.asarray(fc_w, np.float32)
    fc_b = np.asarray(fc_b, np.float32)
    trans = np.asarray(transitions, np.float32)
    start = np.asarray(start_transitions, np.float32)
    end = np.asarray(end_transitions, np.float32)

    prep = _host_prep(embedding, fc_w, fc_b, trans, start)

    if "merged" not in _CACHE:
        ncm = build_merged_kernel()
        ncm.finalize()
        _CACHE["merged"] = ncm

    emb_pad_T = np.zeros((E, VPAD), np.float32)
    emb_pad_T[:, :V] = embedding.T

    # permute tokens so G comes out segment-major: position c holds token
    # sigma(c) = (c % S)*L + c//S (inverse of t -> (t%L)*S + t//L)
    sigma = (np.arange(T) % S) * L + np.arange(T) // S
    x_perm = x[:, sigma]
    tags_m = np.where(x_perm != 0, tags[:, sigma], C).astype(NPBF)

    # map vocab row v to its row index in the packed AllGather layout:
    # shard k = v//6400 stores row v at (v%128)*50 + (v%6400)//128
    def pk(v):
        k = v // VSH
        r = v % VSH
        return k * VSH + (r % 128) * NTILE + r // 128

    in_maps = []
    for k in range(NCORES):
        sl = slice(k * BL, (k + 1) * BL)
        xt = pk(x_perm[sl]).reshape(BL, T // 128, 128).transpose(2, 1, 0) \
                           .reshape(128, T // 128 * BL).astype(np.int32)
        in_maps.append({
            "embT_s": np.ascontiguousarray(emb_pad_T[:, k * VSH:(k + 1) * VSH]),
            "fc_w": fc_w,
            "x_t": np.ascontiguousarray(xt),
            "tags_f": np.ascontiguousarray(tags_m[sl]),
            "blockP": prep["blockP"], "blockPT": prep["blockPT"],
            "bcast8": prep["bcast8"], "iota_rep": prep["iota_rep"],
        })
    res = _run(_CACHE["merged"], in_maps, "merged")

    # t2_out comes back packed (128, 50*C): row v=i*128+p at [p, i*C:(i+1)*C]
    t2_full = np.concatenate(
        [np.asarray(res[k]["t2_out"]).reshape(128, NTILE, C)
         .transpose(1, 0, 2).reshape(VSH, C) for k in range(NCORES)], axis=0)

    # ---- host combine (float64, vectorized) ----
    lengths = (x != 0).sum(1)                        # (B,)
    start64 = start.astype(np.float64)
    end64 = end.astype(np.float64)
    fcb64 = fc_b.astype(np.float64)
    Pe = prep["P_eff"]                               # (C, C) float64
    t264 = t2_full.astype(np.float64)                # (VPAD, C)
    exp_end = np.exp(end64)

    em_total = sum(float(np.asarray(res[k]["num_out"], np.float64).sum())
                   for k in range(NCORES))
    r = np.concatenate(
        [np.asarray(res[k]["r_out"]).astype(np.float64).reshape(BL, C, S)
         for k in range(NCORES)], axis=0)            # (B, C, S)
    d = np.concatenate(
        [np.asarray(res[k]["d_out"]).astype(np.float64).reshape(BL, C, S)
         for k in range(NCORES)], axis=0)            # (B, C, S)

    num = start64[tags[:, 0]] + fcb64[tags[:, 0]]
    num += end64[tags[np.arange(B), lengths - 1]]
    # transition terms (pure tags/params, no device data)
    maskf = (x[:, 1:] != 0).astype(np.float64)
    num += (prep["trans_n"][tags[:, :-1], tags[:, 1:]] * maskf).sum(axis=1)

    # exact alpha over segment 0 (tokens 0..L-1) replaces device r[:,:,0]
    # (device r0 lacks the start-transition factor)
    alpha0 = np.exp(start64[None, :] + t264[x[:, 0]] + fcb64[None, :])  # (B, C)
    for t in range(1, L):
        w = np.exp(t264[x[:, t]] + fcb64[None, :])
        alpha0 = (alpha0 @ Pe) * w        # lengths >= T//2 > L, so no masking
    r[:, :, 0] = alpha0

    # full-segment junction chain: for s in 1..sstar-1:
    #   logZ += log(r[:,:,s-1] @ (Pe @ d[:,:,s])) - log(r[:,:,s].sum())
    sstar = (lengths - 1) // L                       # (B,)
    cs = np.einsum('cd,bds->bcs', Pe, d)             # (B, C, S)
    t1 = np.einsum('bcs,bcs->bs', r[:, :, :-1], cs[:, :, 1:])   # junctions 1..S-1
    rs = r.sum(axis=1)                               # (B, S)
    s_idx = np.arange(1, S)[None, :]                 # (1, S-1)
    jmask = s_idx <= (sstar[:, None] - 1)            # (B, S-1)
    terms = np.where(jmask, np.log(t1) - np.log(rs[:, 1:]), 0.0)
    logZ = terms.sum(axis=1)                         # (B,)

    # ragged tail: exact alpha recursion from segment sstar-1's r
    alpha = r[np.arange(B), :, sstar - 1].copy()     # (B, C)
    tail_len = lengths - sstar * L                   # in [1, L]
    for t_off in range(L):
        active = t_off < tail_len                    # (B,)
        t_idx = np.minimum(sstar * L + t_off, T - 1)
        w = np.exp(t264[x[np.arange(B), t_idx]] + fcb64[None, :])   # (B, C)
        alpha_new = (alpha @ Pe) * w
        alpha = np.where(active[:, None], alpha_new, alpha)
    logZ += np.log(alpha @ exp_end)

    total = -(num - logZ).sum() - em_total
    return np.array(total, dtype=np.float32)


# revision 51
# speedup vs baseline: 1.4716x; 1.4716x over previous
"""CRF negative-log-likelihood kernel for Trainium2 (8 NeuronCores, batch-sharded).

Algorithm:
  - Launch 1 (vocab-sharded): t2 = embedding @ fc_w in bf16. Host pre-transposes
    the embedding shard so the kernel is just convert-to-bf16 + 50 matmuls
    (lhsT = embT chunk, rhs = fc_w), no PE transposes. Output t2 is bf16
    (32B rows) to halve gather traffic.
  - Launch 2 (batch-sharded, 8 rows/core, bf16 compute): merged indirect-DMA
    gathers of t2 rows (8 calls, 4096 descriptors each), bf16 PE-block
    transposes into class-on-partition layout, numerator via one-hot matmul +
    multiply-reduce, and a segmented forward/backward scan (L=16 steps, S=256
    segments on the free dim) in linear space with the two scan chains
    interleaved so vector muls hide behind the other chain's matmuls.
  - Host (float64, vectorized): rank-1 junction chain across segments, exact
    partial segment for each row's ragged tail, final scalar assembly.
"""
import sys
sys.path.insert(0, "/opt/trn_rl_repo")
import numpy as np
import ml_dtypes
from contextlib import ExitStack

import concourse.bass as bass
import concourse.bacc as bacc_mod
import concourse.mybir as mybir
import concourse.tile as tile
from concourse.masks import make_identity
from concourse.bass_utils import run_bass_kernel_spmd

F32 = mybir.dt.float32
BF16 = mybir.dt.bfloat16
I32 = mybir.dt.int32
NPBF = ml_dtypes.bfloat16

V, E, C = 50257, 128, 16
B, T = 64, 4096
L, S = 8, 512
VPAD = 51200
VSH = VPAD // 8
BL = 8
NCHUNK = 8
CHW = T // NCHUNK
NCORES = 8

LAST_EXEC_NS = {}
_TRACE = False
_CACHE = {}


def build_t2_kernel():
    nc = bacc_mod.Bacc()
    # embT_s: host-pretransposed shard, (E, VSH) f32
    embT_s = nc.dram_tensor("embT_s", [E, VSH], F32, kind="ExternalInput")
    fc_w = nc.dram_tensor("fc_w", [E, C], F32, kind="ExternalInput")
    # t2 shard TRANSPOSED: (C, VSH); host un-transposes. One stationary fcw,
    # wide matmuls (out 16 x 640), contiguous out-DMA.
    t2_s = nc.dram_tensor("t2_s", [C, VSH], BF16, kind="ExternalOutput")

    NGRP = 10                   # DMA/convert/matmul granularity
    GW = VSH // NGRP            # 640 columns per group
    with ExitStack() as ctx:
        tc = ctx.enter_context(tile.TileContext(nc))
        singles = ctx.enter_context(tc.tile_pool(name="singles", bufs=1))
        psum = ctx.enter_context(tc.tile_pool(name="psum", bufs=4, space="PSUM"))

        fcw_f32 = singles.tile([E, C], F32)
        nc.scalar.dma_start(out=fcw_f32[:], in_=fc_w[:])
        fcw_bf = singles.tile([E, C], BF16)
        nc.vector.tensor_copy(fcw_bf[:], fcw_f32[:])

        EMBT = singles.tile([128, VSH], F32)
        EMBTb = singles.tile([128, VSH], BF16)
        T2T = singles.tile([C, VSH], BF16)
        MW = GW // 2            # 320 f32 fits a 2KB PSUM bank
        for g in range(NGRP):
            eng = nc.sync if g % 2 == 0 else nc.scalar
            eng.dma_start(out=EMBT[:, g * GW:(g + 1) * GW],
                          in_=embT_s[:, g * GW:(g + 1) * GW])
            for h in range(2):
                m0 = g * GW + h * MW
                if h == 0:
                    nc.vector.tensor_copy(EMBTb[:, m0:m0 + MW],
                                          EMBT[:, m0:m0 + MW])
                else:
                    nc.scalar.copy(EMBTb[:, m0:m0 + MW],
                                   EMBT[:, m0:m0 + MW])
        for m in range(2 * NGRP):
            ps2 = psum.tile([C, MW], F32, tag="p2")
            nc.tensor.matmul(ps2[:], lhsT=fcw_bf[:],
                             rhs=EMBTb[:, m * MW:(m + 1) * MW],
                             start=True, stop=True)
            if m % 2 == 0:
                nc.vector.tensor_copy(T2T[:, m * MW:(m + 1) * MW], ps2[:])
            else:
                nc.scalar.copy(T2T[:, m * MW:(m + 1) * MW], ps2[:])
            if m == NGRP - 1:
                nc.sync.dma_start(out=t2_s[:, :NGRP * MW],
                                  in_=T2T[:, :NGRP * MW])
        nc.scalar.dma_start(out=t2_s[:, NGRP * MW:], in_=T2T[:, NGRP * MW:])
    return nc


def _strided(base_ap, k, step, count):
    return bass.AP(tensor=base_ap.tensor, offset=base_ap.offset + k,
                   ap=[base_ap.ap[0], [step, count]])


def build_main_kernel():
    nc = bacc_mod.Bacc()
    x_t = nc.dram_tensor("x_t", [128, T // 128 * BL], I32, kind="ExternalInput")
    tags_f = nc.dram_tensor("tags_f", [BL, T], BF16, kind="ExternalInput")
    t2 = nc.dram_tensor("t2", [VPAD, C], BF16, kind="ExternalInput")
    blockP = nc.dram_tensor("blockP", [128, 128], BF16, kind="ExternalInput")
    blockPT = nc.dram_tensor("blockPT", [128, 128], BF16, kind="ExternalInput")
    bcast8 = nc.dram_tensor("bcast8", [BL, 128], BF16, kind="ExternalInput")
    iota_rep = nc.dram_tensor("iota_rep", [128, CHW], BF16, kind="ExternalInput")

    r_out = nc.dram_tensor("r_out", [128, S], BF16, kind="ExternalOutput")
    d_out = nc.dram_tensor("d_out", [128, S], BF16, kind="ExternalOutput")
    num_out = nc.dram_tensor("num_out", [1, 2 * NCHUNK], F32, kind="ExternalOutput")

    with ExitStack() as ctx:
        tc = ctx.enter_context(tile.TileContext(nc))
        singles = ctx.enter_context(tc.tile_pool(name="singles", bufs=1))
        big = ctx.enter_context(tc.tile_pool(name="big", bufs=1))
        scratch = ctx.enter_context(tc.tile_pool(name="scratch", bufs=3))
        psum = ctx.enter_context(tc.tile_pool(name="psum", bufs=3, space="PSUM"))
        psumT = ctx.enter_context(tc.tile_pool(name="psumT", bufs=2, space="PSUM"))
        psum2 = ctx.enter_context(tc.tile_pool(name="psum2", bufs=1, space="PSUM"))

        # input DMAs spread across queues for parallel issue
        xt_sb = singles.tile([128, T // 128 * BL], I32)
        nc.sync.dma_start(out=xt_sb[:], in_=x_t[:])
        tagsf_sb = singles.tile([BL, T], BF16)
        nc.scalar.dma_start(out=tagsf_sb[:], in_=tags_f[:])
        bcast8_sb = singles.tile([BL, 128], BF16)
        nc.scalar.dma_start(out=bcast8_sb[:], in_=bcast8[:])
        iotar_sb = singles.tile([128, CHW], BF16)
        nc.scalar.dma_start(out=iotar_sb[:], in_=iota_rep[:])
        blockP_sb = singles.tile([128, 128], BF16)
        nc.scalar.dma_start(out=blockP_sb[:], in_=blockP[:])
        blockPT_sb = singles.tile([128, 128], BF16)
        nc.scalar.dma_start(out=blockPT_sb[:], in_=blockPT[:])

        TM = big.tile([128, T], BF16)
        G = big.tile([128, T], BF16)
        EXPG = big.tile([128, T], BF16)
        W = big.tile([128, T], BF16)
        num_sb = singles.tile([1, 2 * NCHUNK], F32)

        nc.vector.memset(num_sb[:], 0.0)

        EXPGap = EXPG[:]
        Gap = G[:]

        ones_sb = singles.tile([128, 1], BF16)
        nc.vector.memset(ones_sb[:], 1.0)
        ident_bf = singles.tile([128, 128], BF16)
        make_identity(nc, ident_bf[:])
        r_sb = big.tile([128, S], BF16)
        nc.vector.memset(r_sb[:], 1.0)
        d_sb = big.tile([128, S], BF16)

        # Host permutes the token order so that G comes out SEGMENT-MAJOR:
        # G column k*S + s = token s*L + k. Chunk c of the gather therefore
        # holds exactly scan step k=c's emission block, so the forward scan
        # chases the gather; with the balanced gather order below the
        # backward scan chases from the other end.
        GORDER = [0, 7, 1, 6, 2, 5, 3, 4]

        # --- numerator W build first: only needs tags, runs during gather ---
        for c in GORDER:
            c0 = c * CHW
            psA = psum.tile([128, CHW], F32, tag="ps")
            nc.tensor.matmul(psA[:], lhsT=bcast8_sb[:],
                             rhs=tagsf_sb[:, c0:c0 + CHW], start=True, stop=True)
            nc.vector.tensor_tensor(out=W[:, c0:c0 + CHW], in0=psA[:],
                                    in1=iotar_sb[:], op=mybir.AluOpType.is_equal)

        # --- gather: one merged indirect DMA per chunk (4096 descriptors).
        # Offsets enumerate (partition, col) C-order; each offset owns 16
        # contiguous bf16 of the dest view.
        ncc = CHW // 16
        scr2s = {}
        for c in GORDER:
            c0 = c * CHW
            nc.gpsimd.indirect_dma_start(
                out=TM[:, c0:c0 + CHW],
                out_offset=None,
                in_=t2[:],
                in_offset=bass.IndirectOffsetOnAxis(
                    ap=xt_sb[:, c * ncc:(c + 1) * ncc], axis=0),
            )

        # per-chunk pipeline pieces, issued in gather-slot order below
        nbl = CHW // 128
        psN = psum2.tile([1, CHW], F32, tag="psN")

        def xbar_exp(c):
            # PE block-transposes (keeps the DMA engines free for gather
            # descriptors); copy back on DVE (2x bf16 mode)
            c0 = c * CHW
            psT = psumT.tile([128, CHW], BF16, tag="psT")
            for b in range(nbl):
                nc.tensor.transpose(psT[:, b * 128:(b + 1) * 128],
                                    TM[:, c0 + b * 128:c0 + (b + 1) * 128],
                                    ident_bf[:])
            nc.vector.tensor_copy(G[:, c0:c0 + CHW], psT[:])
            nc.scalar.activation(EXPG[:, c0:c0 + CHW], G[:, c0:c0 + CHW],
                                 mybir.ActivationFunctionType.Exp)

        ACC_ORDER = [0, 7, 1, 6, 2, 5, 3, 4]

        def numer(c):
            # em_tag contribution: sum(G * W) via DVE mul (2x bf16 mode)
            # + ones-matmul accumulation into psN
            c0 = c * CHW
            scr2 = scratch.tile([128, CHW], BF16, tag=f"scr2_{c % 3}")
            nc.vector.tensor_mul(scr2[:], G[:, c0:c0 + CHW], W[:, c0:c0 + CHW])
            i = ACC_ORDER.index(c)
            nc.tensor.matmul(psN[:], lhsT=ones_sb[:], rhs=scr2[:],
                             start=(i == 0), stop=(i == NCHUNK - 1))

        def fw(k):
            psR = psum2.tile([128, S], F32, tag="psR")
            nc.tensor.matmul(psR[:], lhsT=blockP_sb[:], rhs=r_sb[:],
                             start=True, stop=True)
            nc.vector.tensor_mul(r_sb[:], psR[:], EXPG[:, k * S:(k + 1) * S])

        def bw(k):
            psD = psum2.tile([128, S], F32, tag="psD")
            nc.tensor.matmul(psD[:], lhsT=blockPT_sb[:], rhs=d_sb[:],
                             start=True, stop=True)
            nc.vector.tensor_mul(d_sb[:], psD[:], EXPG[:, k * S:(k + 1) * S])

        def dinit():
            nc.vector.tensor_copy(d_sb[:], EXPG[:, (L - 1) * S:L * S])

        # slot-ordered issue: each op runs as its chunk lands
        xbar_exp(0); fw(0); numer(0)
        xbar_exp(7); dinit(); numer(7)
        xbar_exp(1); fw(1); numer(1)
        xbar_exp(6); bw(6); numer(6)
        xbar_exp(2); fw(2); numer(2)
        xbar_exp(5); bw(5); numer(5)
        xbar_exp(3); fw(3); numer(3)
        xbar_exp(4); fw(4); numer(4)
        bw(4)
        fw(5)
        bw(3)
        fw(6)
        bw(2)
        fw(7)
        bw(1)
        bw(0)
        nc.vector.reduce_sum(out=num_sb[:, 0:1], in_=psN[:],
                             axis=mybir.AxisListType.X)

        nc.sync.dma_start(out=r_out[:], in_=r_sb[:])
        nc.sync.dma_start(out=num_out[:], in_=num_sb[:])
        nc.sync.dma_start(out=d_out[:], in_=d_sb[:])
    return nc


def _host_prep(embedding, fc_w, fc_b, trans, start):
    P_eff64 = np.exp(trans.astype(np.float64) + fc_b[None, :].astype(np.float64))
    trans_n = (trans + fc_b[None, :]).astype(np.float32)
    P_eff32 = P_eff64.astype(np.float32)

    eye8 = np.eye(BL, dtype=np.float32)
    return dict(
        P_eff=P_eff64,
        trans_n=trans_n.astype(np.float64),
        blockP=np.ascontiguousarray(np.kron(eye8, P_eff32)).astype(NPBF),
        blockPT=np.ascontiguousarray(np.kron(eye8, P_eff32.T.copy())).astype(NPBF),
        bcast8=np.ascontiguousarray(np.kron(eye8, np.ones((1, C), np.float32))).astype(NPBF),
        iota_rep=np.ascontiguousarray(
            np.tile(np.tile(np.arange(C, dtype=np.float32), BL)[:, None],
                    (1, CHW))).astype(NPBF),
    )


LAST_RESULTS = {}


def _run(nc, in_maps, label):
    res = run_bass_kernel_spmd(nc, in_maps, core_ids=list(range(NCORES)),
                               trace=_TRACE)
    if res.exec_time_ns is not None:
        LAST_EXEC_NS[label] = res.exec_time_ns
    LAST_RESULTS[label] = res
    return res.results


def kernel(x, tags, embedding, fc_w, fc_b, start_transitions, end_transitions,
           transitions):
    x = np.asarray(x, np.int32)
    tags = np.asarray(tags, np.int32)
    embedding = np.asarray(embedding, np.float32)
    fc_w = np.asarray(fc_w, np.float32)
    fc_b = np.asarray(fc_b, np.float32)
    trans = np.asarray(transitions, np.float32)
    start = np.asarray(start_transitions, np.float32)
    end = np.asarray(end_transitions, np.float32)

    prep = _host_prep(embedding, fc_w, fc_b, trans, start)

    if "t2" not in _CACHE:
        nc1 = build_t2_kernel()
        nc1.finalize()
        _CACHE["t2"] = nc1
    if "main" not in _CACHE:
        nc2 = build_main_kernel()
        nc2.finalize()
        _CACHE["main"] = nc2

    # ---- launch 1: t2 = emb_pad @ fc_w (bf16 out), vocab-sharded ----
    emb_pad_T = np.zeros((E, VPAD), np.float32)
    emb_pad_T[:, :V] = embedding.T
    in1 = [{"embT_s": np.ascontiguousarray(emb_pad_T[:, k * VSH:(k + 1) * VSH]),
            "fc_w": fc_w} for k in range(NCORES)]
    res1 = _run(_CACHE["t2"], in1, "t2")
    # t2_s comes back transposed (C, VSH)
    t2_full = np.concatenate(
        [np.asarray(res1[k]["t2_s"]).T for k in range(NCORES)], axis=0)
    t2_full = np.ascontiguousarray(t2_full)          # (VPAD, C) bf16

    # ---- launch 2: main kernel, batch-sharded ----
    # permute tokens so G comes out segment-major: position c holds token
    # sigma(c) = (c % S)*L + c//S  (inverse of t -> (t%L)*S + t//L)
    sigma = (np.arange(T) % S) * L + np.arange(T) // S
    x_perm = x[:, sigma]
    tags_m = np.where(x_perm != 0, tags[:, sigma], C).astype(NPBF)
    in2 = []
    for k in range(NCORES):
        sl = slice(k * BL, (k + 1) * BL)
        xt = x_perm[sl].reshape(BL, T // 128, 128).transpose(2, 1, 0) \
                       .reshape(128, T // 128 * BL)
        in2.append({
            "x_t": np.ascontiguousarray(xt),
            "tags_f": np.ascontiguousarray(tags_m[sl]),
            "t2": t2_full,
            "blockP": prep["blockP"], "blockPT": prep["blockPT"],
            "bcast8": prep["bcast8"],
            "iota_rep": prep["iota_rep"],
        })
    res2 = _run(_CACHE["main"], in2, "main")

    # ---- host combine (float64, vectorized) ----
    lengths = (x != 0).sum(1)                        # (B,)
    start64 = start.astype(np.float64)
    end64 = end.astype(np.float64)
    fcb64 = fc_b.astype(np.float64)
    Pe = prep["P_eff"]                               # (C, C) float64
    t264 = t2_full.astype(np.float64)                # (VPAD, C)
    exp_end = np.exp(end64)

    em_total = sum(float(np.asarray(res2[k]["num_out"], np.float64).sum())
                   for k in range(NCORES))
    r = np.concatenate(
        [np.asarray(res2[k]["r_out"]).astype(np.float64).reshape(BL, C, S)
         for k in range(NCORES)], axis=0)            # (B, C, S)
    d = np.concatenate(
        [np.asarray(res2[k]["d_out"]).astype(np.float64).reshape(BL, C, S)
         for k in range(NCORES)], axis=0)            # (B, C, S)

    num = start64[tags[:, 0]] + fcb64[tags[:, 0]]
    num += end64[tags[np.arange(B), lengths - 1]]
    # transition terms (pure tags/params, no device data)
    maskf = (x[:, 1:] != 0).astype(np.float64)
    num += (prep["trans_n"][tags[:, :-1], tags[:, 1:]] * maskf).sum(axis=1)

    # exact alpha over segment 0 (tokens 0..L-1) replaces device r[:,:,0]
    # (device r0 lacks the start-transition factor)
    alpha0 = np.exp(start64[None, :] + t264[x[:, 0]] + fcb64[None, :])  # (B, C)
    for t in range(1, L):
        w = np.exp(t264[x[:, t]] + fcb64[None, :])
        alpha0 = (alpha0 @ Pe) * w        # lengths >= T//2 > L, so no masking
    r[:, :, 0] = alpha0

    # full-segment junction chain: for s in 1..sstar-1:
    #   logZ += log(r[:,:,s-1] @ (Pe @ d[:,:,s])) - log(r[:,:,s].sum())
    sstar = (lengths - 1) // L                       # (B,)
    cs = np.einsum('cd,bds->bcs', Pe, d)             # (B, C, S)
    t1 = np.einsum('bcs,bcs->bs', r[:, :, :-1], cs[:, :, 1:])   # junction at s=1..S-1
    rs = r.sum(axis=1)                               # (B, S)
    s_idx = np.arange(1, S)[None, :]                 # (1, S-1)
    jmask = s_idx <= (sstar[:, None] - 1)            # (B, S-1)
    terms = np.where(jmask, np.log(t1) - np.log(rs[:, 1:]), 0.0)
    logZ = terms.sum(axis=1)                         # (B,)

    # ragged tail: exact alpha recursion from segment sstar-1's r
    alpha = r[np.arange(B), :, sstar - 1].copy()     # (B, C)
    tail_len = lengths - sstar * L                   # in [1, L]
    for t_off in range(L):
        active = t_off < tail_len                    # (B,)
        t_idx = np.minimum(sstar * L + t_off, T - 1)
        w = np.exp(t264[x[np.arange(B), t_idx]] + fcb64[None, :])   # (B, C)
        alpha_new = (alpha @ Pe) * w
        alpha = np.where(active[:, None], alpha_new, alpha)
    logZ += np.log(alpha @ exp_end)

    total = -(num - logZ).sum() - em_total
    return np.array(total, dtype=np.float32)


# revision 52
# speedup vs baseline: 1.4814x; 1.0067x over previous
"""CRF negative-log-likelihood kernel for Trainium2 (8 NeuronCores, batch-sharded).

Algorithm:
  - Launch 1 (vocab-sharded): t2 = embedding @ fc_w in bf16. Host pre-transposes
    the embedding shard so the kernel is just convert-to-bf16 + 50 matmuls
    (lhsT = embT chunk, rhs = fc_w), no PE transposes. Output t2 is bf16
    (32B rows) to halve gather traffic.
  - Launch 2 (batch-sharded, 8 rows/core, bf16 compute): merged indirect-DMA
    gathers of t2 rows (8 calls, 4096 descriptors each), bf16 PE-block
    transposes into class-on-partition layout, numerator via one-hot matmul +
    multiply-reduce, and a segmented forward/backward scan (L=16 steps, S=256
    segments on the free dim) in linear space with the two scan chains
    interleaved so vector muls hide behind the other chain's matmuls.
  - Host (float64, vectorized): rank-1 junction chain across segments, exact
    partial segment for each row's ragged tail, final scalar assembly.
"""
import sys
sys.path.insert(0, "/opt/trn_rl_repo")
import numpy as np
import ml_dtypes
from contextlib import ExitStack

import concourse.bass as bass
import concourse.bacc as bacc_mod
import concourse.mybir as mybir
import concourse.tile as tile
from concourse.masks import make_identity
from concourse.bass_utils import run_bass_kernel_spmd

F32 = mybir.dt.float32
BF16 = mybir.dt.bfloat16
I32 = mybir.dt.int32
NPBF = ml_dtypes.bfloat16

V, E, C = 50257, 128, 16
B, T = 64, 4096
L, S = 8, 512
VPAD = 51200
VSH = VPAD // 8
BL = 8
NCHUNK = 8
CHW = T // NCHUNK
NCORES = 8

LAST_EXEC_NS = {}
_TRACE = False
_CACHE = {}


def build_t2_kernel():
    nc = bacc_mod.Bacc()
    # embT_s: host-pretransposed shard, (E, VSH) f32
    embT_s = nc.dram_tensor("embT_s", [E, VSH], F32, kind="ExternalInput")
    fc_w = nc.dram_tensor("fc_w", [E, C], F32, kind="ExternalInput")
    # t2 shard TRANSPOSED: (C, VSH); host un-transposes. One stationary fcw,
    # wide matmuls (out 16 x 640), contiguous out-DMA.
    t2_s = nc.dram_tensor("t2_s", [C, VSH], BF16, kind="ExternalOutput")

    NGRP = 10                   # DMA/convert/matmul granularity
    GW = VSH // NGRP            # 640 columns per group
    with ExitStack() as ctx:
        tc = ctx.enter_context(tile.TileContext(nc))
        singles = ctx.enter_context(tc.tile_pool(name="singles", bufs=1))
        psum = ctx.enter_context(tc.tile_pool(name="psum", bufs=4, space="PSUM"))

        fcw_f32 = singles.tile([E, C], F32)
        nc.scalar.dma_start(out=fcw_f32[:], in_=fc_w[:])
        fcw_bf = singles.tile([E, C], BF16)
        nc.vector.tensor_copy(fcw_bf[:], fcw_f32[:])

        EMBT = singles.tile([128, VSH], F32)
        EMBTb = singles.tile([128, VSH], BF16)
        T2T = singles.tile([C, VSH], BF16)
        MW = GW // 2            # 320 f32 fits a 2KB PSUM bank
        for g in range(NGRP):
            eng = nc.sync if g % 2 == 0 else nc.gpsimd
            eng.dma_start(out=EMBT[:, g * GW:(g + 1) * GW],
                          in_=embT_s[:, g * GW:(g + 1) * GW])
            for h in range(2):
                m0 = g * GW + h * MW
                nc.vector.tensor_copy(EMBTb[:, m0:m0 + MW],
                                      EMBT[:, m0:m0 + MW])
        for m in range(2 * NGRP):
            ps2 = psum.tile([C, MW], F32, tag="p2")
            nc.tensor.matmul(ps2[:], lhsT=fcw_bf[:],
                             rhs=EMBTb[:, m * MW:(m + 1) * MW],
                             start=True, stop=True)
            nc.scalar.copy(T2T[:, m * MW:(m + 1) * MW], ps2[:])
            if m == NGRP - 1:
                nc.sync.dma_start(out=t2_s[:, :NGRP * MW],
                                  in_=T2T[:, :NGRP * MW])
        nc.scalar.dma_start(out=t2_s[:, NGRP * MW:], in_=T2T[:, NGRP * MW:])
    return nc


def _strided(base_ap, k, step, count):
    return bass.AP(tensor=base_ap.tensor, offset=base_ap.offset + k,
                   ap=[base_ap.ap[0], [step, count]])


def build_main_kernel():
    nc = bacc_mod.Bacc()
    x_t = nc.dram_tensor("x_t", [128, T // 128 * BL], I32, kind="ExternalInput")
    tags_f = nc.dram_tensor("tags_f", [BL, T], BF16, kind="ExternalInput")
    t2 = nc.dram_tensor("t2", [VPAD, C], BF16, kind="ExternalInput")
    blockP = nc.dram_tensor("blockP", [128, 128], BF16, kind="ExternalInput")
    blockPT = nc.dram_tensor("blockPT", [128, 128], BF16, kind="ExternalInput")
    bcast8 = nc.dram_tensor("bcast8", [BL, 128], BF16, kind="ExternalInput")
    iota_rep = nc.dram_tensor("iota_rep", [128, CHW], BF16, kind="ExternalInput")

    r_out = nc.dram_tensor("r_out", [128, S], BF16, kind="ExternalOutput")
    d_out = nc.dram_tensor("d_out", [128, S], BF16, kind="ExternalOutput")
    num_out = nc.dram_tensor("num_out", [1, 2 * NCHUNK], F32, kind="ExternalOutput")

    with ExitStack() as ctx:
        tc = ctx.enter_context(tile.TileContext(nc))
        singles = ctx.enter_context(tc.tile_pool(name="singles", bufs=1))
        big = ctx.enter_context(tc.tile_pool(name="big", bufs=1))
        scratch = ctx.enter_context(tc.tile_pool(name="scratch", bufs=3))
        psum = ctx.enter_context(tc.tile_pool(name="psum", bufs=3, space="PSUM"))
        psumT = ctx.enter_context(tc.tile_pool(name="psumT", bufs=2, space="PSUM"))
        psum2 = ctx.enter_context(tc.tile_pool(name="psum2", bufs=1, space="PSUM"))

        # input DMAs spread across queues for parallel issue
        xt_sb = singles.tile([128, T // 128 * BL], I32)
        nc.sync.dma_start(out=xt_sb[:], in_=x_t[:])
        tagsf_sb = singles.tile([BL, T], BF16)
        nc.scalar.dma_start(out=tagsf_sb[:], in_=tags_f[:])
        bcast8_sb = singles.tile([BL, 128], BF16)
        nc.scalar.dma_start(out=bcast8_sb[:], in_=bcast8[:])
        iotar_sb = singles.tile([128, CHW], BF16)
        nc.scalar.dma_start(out=iotar_sb[:], in_=iota_rep[:])
        blockP_sb = singles.tile([128, 128], BF16)
        nc.scalar.dma_start(out=blockP_sb[:], in_=blockP[:])
        blockPT_sb = singles.tile([128, 128], BF16)
        nc.scalar.dma_start(out=blockPT_sb[:], in_=blockPT[:])

        TM = big.tile([128, T], BF16)
        G = big.tile([128, T], BF16)
        EXPG = big.tile([128, T], BF16)
        W = big.tile([128, T], BF16)
        num_sb = singles.tile([1, 2 * NCHUNK], F32)

        nc.vector.memset(num_sb[:], 0.0)

        EXPGap = EXPG[:]
        Gap = G[:]

        ones_sb = singles.tile([128, 1], BF16)
        nc.vector.memset(ones_sb[:], 1.0)
        ident_bf = singles.tile([128, 128], BF16)
        make_identity(nc, ident_bf[:])
        r_sb = big.tile([128, S], BF16)
        nc.vector.memset(r_sb[:], 1.0)
        d_sb = big.tile([128, S], BF16)

        # Host permutes the token order so that G comes out SEGMENT-MAJOR:
        # G column k*S + s = token s*L + k. Chunk c of the gather therefore
        # holds exactly scan step k=c's emission block, so the forward scan
        # chases the gather; with the balanced gather order below the
        # backward scan chases from the other end.
        GORDER = [0, 7, 1, 6, 2, 5, 3, 4]

        # --- numerator W build first: only needs tags, runs during gather ---
        for c in GORDER:
            c0 = c * CHW
            psA = psum.tile([128, CHW], F32, tag="ps")
            nc.tensor.matmul(psA[:], lhsT=bcast8_sb[:],
                             rhs=tagsf_sb[:, c0:c0 + CHW], start=True, stop=True)
            nc.vector.tensor_tensor(out=W[:, c0:c0 + CHW], in0=psA[:],
                                    in1=iotar_sb[:], op=mybir.AluOpType.is_equal)

        # --- gather: one merged indirect DMA per chunk (4096 descriptors).
        # Offsets enumerate (partition, col) C-order; each offset owns 16
        # contiguous bf16 of the dest view.
        ncc = CHW // 16
        scr2s = {}
        for c in GORDER:
            c0 = c * CHW
            nc.gpsimd.indirect_dma_start(
                out=TM[:, c0:c0 + CHW],
                out_offset=None,
                in_=t2[:],
                in_offset=bass.IndirectOffsetOnAxis(
                    ap=xt_sb[:, c * ncc:(c + 1) * ncc], axis=0),
            )

        # per-chunk pipeline pieces, issued in gather-slot order below
        nbl = CHW // 128
        psN = psum2.tile([1, CHW], F32, tag="psN")

        def xbar_exp(c):
            # PE block-transposes (keeps the DMA engines free for gather
            # descriptors); copy back on DVE (2x bf16 mode)
            c0 = c * CHW
            psT = psumT.tile([128, CHW], BF16, tag="psT")
            for b in range(nbl):
                nc.tensor.transpose(psT[:, b * 128:(b + 1) * 128],
                                    TM[:, c0 + b * 128:c0 + (b + 1) * 128],
                                    ident_bf[:])
            nc.vector.tensor_copy(G[:, c0:c0 + CHW], psT[:])
            nc.scalar.activation(EXPG[:, c0:c0 + CHW], G[:, c0:c0 + CHW],
                                 mybir.ActivationFunctionType.Exp)

        ACC_ORDER = [0, 7, 1, 6, 2, 5, 3, 4]

        def numer(c):
            # em_tag contribution: sum(G * W) via DVE mul (2x bf16 mode)
            # + ones-matmul accumulation into psN
            c0 = c * CHW
            scr2 = scratch.tile([128, CHW], BF16, tag=f"scr2_{c % 3}")
            nc.vector.tensor_mul(scr2[:], G[:, c0:c0 + CHW], W[:, c0:c0 + CHW])
            i = ACC_ORDER.index(c)
            nc.tensor.matmul(psN[:], lhsT=ones_sb[:], rhs=scr2[:],
                             start=(i == 0), stop=(i == NCHUNK - 1))

        def fw(k):
            psR = psum2.tile([128, S], F32, tag="psR")
            nc.tensor.matmul(psR[:], lhsT=blockP_sb[:], rhs=r_sb[:],
                             start=True, stop=True)
            nc.vector.tensor_mul(r_sb[:], psR[:], EXPG[:, k * S:(k + 1) * S])

        def bw(k):
            psD = psum2.tile([128, S], F32, tag="psD")
            nc.tensor.matmul(psD[:], lhsT=blockPT_sb[:], rhs=d_sb[:],
                             start=True, stop=True)
            nc.vector.tensor_mul(d_sb[:], psD[:], EXPG[:, k * S:(k + 1) * S])

        def dinit():
            nc.vector.tensor_copy(d_sb[:], EXPG[:, (L - 1) * S:L * S])

        # slot-ordered issue: each op runs as its chunk lands
        xbar_exp(0); fw(0); numer(0)
        xbar_exp(7); dinit(); numer(7)
        xbar_exp(1); fw(1); numer(1)
        xbar_exp(6); bw(6); numer(6)
        xbar_exp(2); fw(2); numer(2)
        xbar_exp(5); bw(5); numer(5)
        xbar_exp(3); fw(3); numer(3)
        xbar_exp(4); fw(4); numer(4)
        bw(4)
        fw(5)
        bw(3)
        fw(6)
        bw(2)
        fw(7)
        bw(1)
        bw(0)
        nc.vector.reduce_sum(out=num_sb[:, 0:1], in_=psN[:],
                             axis=mybir.AxisListType.X)

        nc.sync.dma_start(out=r_out[:], in_=r_sb[:])
        nc.sync.dma_start(out=num_out[:], in_=num_sb[:])
        nc.sync.dma_start(out=d_out[:], in_=d_sb[:])
    return nc


def _host_prep(embedding, fc_w, fc_b, trans, start):
    P_eff64 = np.exp(trans.astype(np.float64) + fc_b[None, :].astype(np.float64))
    trans_n = (trans + fc_b[None, :]).astype(np.float32)
    P_eff32 = P_eff64.astype(np.float32)

    eye8 = np.eye(BL, dtype=np.float32)
    return dict(
        P_eff=P_eff64,
        trans_n=trans_n.astype(np.float64),
        blockP=np.ascontiguousarray(np.kron(eye8, P_eff32)).astype(NPBF),
        blockPT=np.ascontiguousarray(np.kron(eye8, P_eff32.T.copy())).astype(NPBF),
        bcast8=np.ascontiguousarray(np.kron(eye8, np.ones((1, C), np.float32))).astype(NPBF),
        iota_rep=np.ascontiguousarray(
            np.tile(np.tile(np.arange(C, dtype=np.float32), BL)[:, None],
                    (1, CHW))).astype(NPBF),
    )


LAST_RESULTS = {}


def _run(nc, in_maps, label):
    res = run_bass_kernel_spmd(nc, in_maps, core_ids=list(range(NCORES)),
                               trace=_TRACE)
    if res.exec_time_ns is not None:
        LAST_EXEC_NS[label] = res.exec_time_ns
    LAST_RESULTS[label] = res
    return res.results


def kernel(x, tags, embedding, fc_w, fc_b, start_transitions, end_transitions,
           transitions):
    x = np.asarray(x, np.int32)
    tags = np.asarray(tags, np.int32)
    embedding = np.asarray(embedding, np.float32)
    fc_w = np.asarray(fc_w, np.float32)
    fc_b = np.asarray(fc_b, np.float32)
    trans = np.asarray(transitions, np.float32)
    start = np.asarray(start_transitions, np.float32)
    end = np.asarray(end_transitions, np.float32)

    prep = _host_prep(embedding, fc_w, fc_b, trans, start)

    if "t2" not in _CACHE:
        nc1 = build_t2_kernel()
        nc1.finalize()
        _CACHE["t2"] = nc1
    if "main" not in _CACHE:
        nc2 = build_main_kernel()
        nc2.finalize()
        _CACHE["main"] = nc2

    # ---- launch 1: t2 = emb_pad @ fc_w (bf16 out), vocab-sharded ----
    emb_pad_T = np.zeros((E, VPAD), np.float32)
    emb_pad_T[:, :V] = embedding.T
    in1 = [{"embT_s": np.ascontiguousarray(emb_pad_T[:, k * VSH:(k + 1) * VSH]),
            "fc_w": fc_w} for k in range(NCORES)]
    res1 = _run(_CACHE["t2"], in1, "t2")
    # t2_s comes back transposed (C, VSH)
    t2_full = np.concatenate(
        [np.asarray(res1[k]["t2_s"]).T for k in range(NCORES)], axis=0)
    t2_full = np.ascontiguousarray(t2_full)          # (VPAD, C) bf16

    # ---- launch 2: main kernel, batch-sharded ----
    # permute tokens so G comes out segment-major: position c holds token
    # sigma(c) = (c % S)*L + c//S  (inverse of t -> (t%L)*S + t//L)
    sigma = (np.arange(T) % S) * L + np.arange(T) // S
    x_perm = x[:, sigma]
    tags_m = np.where(x_perm != 0, tags[:, sigma], C).astype(NPBF)
    in2 = []
    for k in range(NCORES):
        sl = slice(k * BL, (k + 1) * BL)
        xt = x_perm[sl].reshape(BL, T // 128, 128).transpose(2, 1, 0) \
                       .reshape(128, T // 128 * BL)
        in2.append({
            "x_t": np.ascontiguousarray(xt),
            "tags_f": np.ascontiguousarray(tags_m[sl]),
            "t2": t2_full,
            "blockP": prep["blockP"], "blockPT": prep["blockPT"],
            "bcast8": prep["bcast8"],
            "iota_rep": prep["iota_rep"],
        })
    res2 = _run(_CACHE["main"], in2, "main")

    # ---- host combine (float64, vectorized) ----
    lengths = (x != 0).sum(1)                        # (B,)
    start64 = start.astype(np.float64)
    end64 = end.astype(np.float64)
    fcb64 = fc_b.astype(np.float64)
    Pe = prep["P_eff"]                               # (C, C) float64
    t264 = t2_full.astype(np.float64)                # (VPAD, C)
    exp_end = np.exp(end64)

    em_total = sum(float(np.asarray(res2[k]["num_out"], np.float64).sum())
                   for k in range(NCORES))
    r = np.concatenate(
        [np.asarray(res2[k]["r_out"]).astype(np.float64).reshape(BL, C, S)
         for k in range(NCORES)], axis=0)            # (B, C, S)
    d = np.concatenate(
        [np.asarray(res2[k]["d_out"]).astype(np.float64).reshape(BL, C, S)
         for k in range(NCORES)], axis=0)            # (B, C, S)

    num = start64[tags[:, 0]] + fcb64[tags[:, 0]]
    num += end64[tags[np.arange(B), lengths - 1]]
    # transition terms (pure tags/params, no device data)
    maskf = (x[:, 1:] != 0).astype(np.float64)
    num += (prep["trans_n"][tags[:, :-1], tags[:, 1:]] * maskf).sum(axis=1)

    # exact alpha over segment 0 (tokens 0..L-1) replaces device r[:,:,0]
    # (device r0 lacks the start-transition factor)
    alpha0 = np.exp(start64[None, :] + t264[x[:, 0]] + fcb64[None, :])  # (B, C)
    for t in range(1, L):
        w = np.exp(t264[x[:, t]] + fcb64[None, :])
        alpha0 = (alpha0 @ Pe) * w        # lengths >= T//2 > L, so no masking
    r[:, :, 0] = alpha0

    # full-segment junction chain: for s in 1..sstar-1:
    #   logZ += log(r[:,:,s-1] @ (Pe @ d[:,:,s])) - log(r[:,:,s].sum())
    sstar = (lengths - 1) // L                       # (B,)
    cs = np.einsum('cd,bds->bcs', Pe, d)             # (B, C, S)
    t1 = np.einsum('bcs,bcs->bs', r[:, :, :-1], cs[:, :, 1:])   # junction at s=1..S-1
    rs = r.sum(axis=1)                               # (B, S)
    s_idx = np.arange(1, S)[None, :]                 # (1, S-1)
    jmask = s_idx <= (sstar[:, None] - 1)            # (B, S-1)
    terms = np.where(jmask, np.log(t1) - np.log(rs[:, 1:]), 0.0)
    logZ = terms.sum(axis=1)                         # (B,)

    # ragged tail: exact alpha recursion from segment sstar-1's r
    alpha = r[np.arange(B), :, sstar - 1].copy()     # (B, C)
    tail_len = lengths - sstar * L                   # in [1, L]
    for t_off in range(L):
        active = t_off < tail_len                    # (B,)
        t_idx = np.minimum(sstar * L + t_off, T - 1)
        w = np.exp(t264[x[np.arange(B), t_idx]] + fcb64[None, :])   # (B, C)
        alpha_new = (alpha @ Pe) * w
        alpha = np.where(active[:, None], alpha_new, alpha)
    logZ += np.log(alpha @ exp_end)

    total = -(num - logZ).sum() - em_total
    return np.array(total, dtype=np.float32)


# revision 53
# speedup vs baseline: 1.4930x; 1.0078x over previous
"""CRF negative-log-likelihood kernel for Trainium2 (8 NeuronCores, batch-sharded).

Algorithm:
  - Launch 1 (vocab-sharded): t2 = embedding @ fc_w in bf16. Host pre-transposes
    the embedding shard so the kernel is just convert-to-bf16 + 50 matmuls
    (lhsT = embT chunk, rhs = fc_w), no PE transposes. Output t2 is bf16
    (32B rows) to halve gather traffic.
  - Launch 2 (batch-sharded, 8 rows/core, bf16 compute): merged indirect-DMA
    gathers of t2 rows (8 calls, 4096 descriptors each), bf16 PE-block
    transposes into class-on-partition layout, numerator via one-hot matmul +
    multiply-reduce, and a segmented forward/backward scan (L=16 steps, S=256
    segments on the free dim) in linear space with the two scan chains
    interleaved so vector muls hide behind the other chain's matmuls.
  - Host (float64, vectorized): rank-1 junction chain across segments, exact
    partial segment for each row's ragged tail, final scalar assembly.
"""
import sys
sys.path.insert(0, "/opt/trn_rl_repo")
import numpy as np
import ml_dtypes
from contextlib import ExitStack

import concourse.bass as bass
import concourse.bacc as bacc_mod
import concourse.mybir as mybir
import concourse.tile as tile
from concourse.masks import make_identity
from concourse.bass_utils import run_bass_kernel_spmd

F32 = mybir.dt.float32
BF16 = mybir.dt.bfloat16
I32 = mybir.dt.int32
NPBF = ml_dtypes.bfloat16

V, E, C = 50257, 128, 16
B, T = 64, 4096
L, S = 8, 512
VPAD = 51200
VSH = VPAD // 8
BL = 8
NCHUNK = 8
CHW = T // NCHUNK
NCORES = 8

LAST_EXEC_NS = {}
_TRACE = False
_CACHE = {}


def build_t2_kernel():
    nc = bacc_mod.Bacc()
    # embT_s: host-pretransposed shard, (E, VSH) f32
    embT_s = nc.dram_tensor("embT_s", [E, VSH], F32, kind="ExternalInput")
    fc_w = nc.dram_tensor("fc_w", [E, C], F32, kind="ExternalInput")
    # t2 shard TRANSPOSED: (C, VSH); host un-transposes. One stationary fcw,
    # wide matmuls (out 16 x 640), contiguous out-DMA.
    t2_s = nc.dram_tensor("t2_s", [C, VSH], BF16, kind="ExternalOutput")

    NGRP = 10                   # DMA/convert/matmul granularity
    GW = VSH // NGRP            # 640 columns per group
    with ExitStack() as ctx:
        tc = ctx.enter_context(tile.TileContext(nc))
        singles = ctx.enter_context(tc.tile_pool(name="singles", bufs=1))
        psum = ctx.enter_context(tc.tile_pool(name="psum", bufs=4, space="PSUM"))

        fcw_f32 = singles.tile([E, C], F32)
        nc.scalar.dma_start(out=fcw_f32[:], in_=fc_w[:])
        fcw_bf = singles.tile([E, C], BF16)
        nc.vector.tensor_copy(fcw_bf[:], fcw_f32[:])

        EMBT = singles.tile([128, VSH], F32)
        EMBTb = singles.tile([128, VSH], BF16)
        T2T = singles.tile([C, VSH], BF16)
        MW = GW // 2            # 320 f32 fits a 2KB PSUM bank
        for g in range(NGRP):
            eng = nc.sync if g % 2 == 0 else nc.gpsimd
            eng.dma_start(out=EMBT[:, g * GW:(g + 1) * GW],
                          in_=embT_s[:, g * GW:(g + 1) * GW])
            for h in range(2):
                m0 = g * GW + h * MW
                nc.vector.tensor_copy(EMBTb[:, m0:m0 + MW],
                                      EMBT[:, m0:m0 + MW])
        for m in range(2 * NGRP):
            ps2 = psum.tile([C, MW], F32, tag="p2")
            nc.tensor.matmul(ps2[:], lhsT=fcw_bf[:],
                             rhs=EMBTb[:, m * MW:(m + 1) * MW],
                             start=True, stop=True)
            nc.scalar.copy(T2T[:, m * MW:(m + 1) * MW], ps2[:])
            if m == NGRP - 1:
                nc.sync.dma_start(out=t2_s[:, :NGRP * MW],
                                  in_=T2T[:, :NGRP * MW])
        nc.scalar.dma_start(out=t2_s[:, NGRP * MW:], in_=T2T[:, NGRP * MW:])
    return nc


def _strided(base_ap, k, step, count):
    return bass.AP(tensor=base_ap.tensor, offset=base_ap.offset + k,
                   ap=[base_ap.ap[0], [step, count]])


def build_main_kernel():
    nc = bacc_mod.Bacc()
    x_t = nc.dram_tensor("x_t", [128, T // 128 * BL], I32, kind="ExternalInput")
    tags_f = nc.dram_tensor("tags_f", [BL, T], BF16, kind="ExternalInput")
    t2 = nc.dram_tensor("t2", [VPAD, C], BF16, kind="ExternalInput")
    blockP = nc.dram_tensor("blockP", [128, 128], BF16, kind="ExternalInput")
    blockPT = nc.dram_tensor("blockPT", [128, 128], BF16, kind="ExternalInput")
    bcast8 = nc.dram_tensor("bcast8", [BL, 128], BF16, kind="ExternalInput")
    iota_rep = nc.dram_tensor("iota_rep", [128, CHW], BF16, kind="ExternalInput")

    r_out = nc.dram_tensor("r_out", [128, S], BF16, kind="ExternalOutput")
    d_out = nc.dram_tensor("d_out", [128, S], BF16, kind="ExternalOutput")
    num_out = nc.dram_tensor("num_out", [1, 2 * NCHUNK], F32, kind="ExternalOutput")

    with ExitStack() as ctx:
        tc = ctx.enter_context(tile.TileContext(nc))
        singles = ctx.enter_context(tc.tile_pool(name="singles", bufs=1))
        big = ctx.enter_context(tc.tile_pool(name="big", bufs=1))
        scratch = ctx.enter_context(tc.tile_pool(name="scratch", bufs=3))
        psum = ctx.enter_context(tc.tile_pool(name="psum", bufs=3, space="PSUM"))
        psumT = ctx.enter_context(tc.tile_pool(name="psumT", bufs=2, space="PSUM"))
        psum2 = ctx.enter_context(tc.tile_pool(name="psum2", bufs=1, space="PSUM"))

        # input DMAs spread across queues for parallel issue
        xt_sb = singles.tile([128, T // 128 * BL], I32)
        nc.sync.dma_start(out=xt_sb[:], in_=x_t[:])
        tagsf_sb = singles.tile([BL, T], BF16)
        nc.scalar.dma_start(out=tagsf_sb[:], in_=tags_f[:])
        bcast8_sb = singles.tile([BL, 128], BF16)
        nc.scalar.dma_start(out=bcast8_sb[:], in_=bcast8[:])
        iotar_sb = singles.tile([128, CHW], BF16)
        nc.scalar.dma_start(out=iotar_sb[:], in_=iota_rep[:])
        blockP_sb = singles.tile([128, 128], BF16)
        nc.scalar.dma_start(out=blockP_sb[:], in_=blockP[:])
        blockPT_sb = singles.tile([128, 128], BF16)
        nc.scalar.dma_start(out=blockPT_sb[:], in_=blockPT[:])

        TM = big.tile([128, T], BF16)
        G = big.tile([128, T], BF16)
        EXPG = big.tile([128, T], BF16)
        W = big.tile([128, T], BF16)
        num_sb = singles.tile([1, 2 * NCHUNK], F32)

        nc.vector.memset(num_sb[:], 0.0)

        EXPGap = EXPG[:]
        Gap = G[:]

        ones_sb = singles.tile([128, 1], BF16)
        nc.vector.memset(ones_sb[:], 1.0)
        ident_bf = singles.tile([128, 128], BF16)
        make_identity(nc, ident_bf[:])
        r_sb = big.tile([128, S], BF16)
        nc.vector.memset(r_sb[:], 1.0)
        d_sb = big.tile([128, S], BF16)

        # Host permutes the token order so that G comes out SEGMENT-MAJOR:
        # G column k*S + s = token s*L + k. Chunk c of the gather therefore
        # holds exactly scan step k=c's emission block, so the forward scan
        # chases the gather; with the balanced gather order below the
        # backward scan chases from the other end.
        GORDER = [0, 7, 1, 6, 2, 5, 3, 4]

        # --- numerator W build first: only needs tags, runs during gather ---
        for c in GORDER:
            c0 = c * CHW
            psA = psum.tile([128, CHW], F32, tag="ps")
            nc.tensor.matmul(psA[:], lhsT=bcast8_sb[:],
                             rhs=tagsf_sb[:, c0:c0 + CHW], start=True, stop=True)
            nc.vector.tensor_tensor(out=W[:, c0:c0 + CHW], in0=psA[:],
                                    in1=iotar_sb[:], op=mybir.AluOpType.is_equal)

        # --- gather: one merged indirect DMA per chunk (4096 descriptors).
        # Offsets enumerate (partition, col) C-order; each offset owns 16
        # contiguous bf16 of the dest view.
        ncc = CHW // 16
        scr2s = {}
        for c in GORDER:
            c0 = c * CHW
            nc.gpsimd.indirect_dma_start(
                out=TM[:, c0:c0 + CHW],
                out_offset=None,
                in_=t2[:],
                in_offset=bass.IndirectOffsetOnAxis(
                    ap=xt_sb[:, c * ncc:(c + 1) * ncc], axis=0),
            )

        # per-chunk pipeline pieces, issued in gather-slot order below
        nbl = CHW // 128
        psN = psum2.tile([1, CHW], F32, tag="psN")

        def xbar_exp(c):
            # PE block-transposes (keeps the DMA engines free for gather
            # descriptors); copy back on DVE (2x bf16 mode)
            c0 = c * CHW
            psT = psumT.tile([128, CHW], BF16, tag="psT")
            for b in range(nbl):
                nc.tensor.transpose(psT[:, b * 128:(b + 1) * 128],
                                    TM[:, c0 + b * 128:c0 + (b + 1) * 128],
                                    ident_bf[:])
            nc.vector.tensor_copy(G[:, c0:c0 + CHW], psT[:])
            nc.scalar.activation(EXPG[:, c0:c0 + CHW], G[:, c0:c0 + CHW],
                                 mybir.ActivationFunctionType.Exp)

        ACC_ORDER = [0, 7, 1, 6, 2, 5, 3, 4]

        def numer(c):
            # em_tag contribution: sum(G * W) via DVE mul (2x bf16 mode)
            # + ones-matmul accumulation into psN
            c0 = c * CHW
            scr2 = scratch.tile([128, CHW], BF16, tag=f"scr2_{c % 3}")
            nc.vector.tensor_mul(scr2[:], G[:, c0:c0 + CHW], W[:, c0:c0 + CHW])
            i = ACC_ORDER.index(c)
            nc.tensor.matmul(psN[:], lhsT=ones_sb[:], rhs=scr2[:],
                             start=(i == 0), stop=(i == NCHUNK - 1))

        def fw(k):
            psR = psum2.tile([128, S], F32, tag="psR")
            nc.tensor.matmul(psR[:], lhsT=blockP_sb[:], rhs=r_sb[:],
                             start=True, stop=True)
            nc.vector.tensor_mul(r_sb[:], psR[:], EXPG[:, k * S:(k + 1) * S])

        def bw(k):
            psD = psum2.tile([128, S], F32, tag="psD")
            nc.tensor.matmul(psD[:], lhsT=blockPT_sb[:], rhs=d_sb[:],
                             start=True, stop=True)
            nc.vector.tensor_mul(d_sb[:], psD[:], EXPG[:, k * S:(k + 1) * S])

        def dinit():
            nc.vector.tensor_copy(d_sb[:], EXPG[:, (L - 1) * S:L * S])

        # slot-ordered issue: each op runs as its chunk lands
        xbar_exp(0); fw(0); numer(0)
        xbar_exp(7); dinit(); numer(7)
        xbar_exp(1); fw(1); numer(1)
        xbar_exp(6); bw(6); numer(6)
        xbar_exp(2); fw(2); numer(2)
        xbar_exp(5); bw(5); numer(5)
        xbar_exp(3); fw(3); numer(3)
        xbar_exp(4); fw(4); numer(4)
        bw(4)
        fw(5)
        bw(3)
        fw(6)
        bw(2)
        fw(7)
        bw(1)
        bw(0)
        nc.vector.reduce_sum(out=num_sb[:, 0:1], in_=psN[:],
                             axis=mybir.AxisListType.X)

        nc.sync.dma_start(out=r_out[:], in_=r_sb[:])
        nc.sync.dma_start(out=num_out[:], in_=num_sb[:])
        nc.scalar.dma_start(out=d_out[:], in_=d_sb[:])
    return nc


def _host_prep(embedding, fc_w, fc_b, trans, start):
    P_eff64 = np.exp(trans.astype(np.float64) + fc_b[None, :].astype(np.float64))
    trans_n = (trans + fc_b[None, :]).astype(np.float32)
    P_eff32 = P_eff64.astype(np.float32)

    eye8 = np.eye(BL, dtype=np.float32)
    return dict(
        P_eff=P_eff64,
        trans_n=trans_n.astype(np.float64),
        blockP=np.ascontiguousarray(np.kron(eye8, P_eff32)).astype(NPBF),
        blockPT=np.ascontiguousarray(np.kron(eye8, P_eff32.T.copy())).astype(NPBF),
        bcast8=np.ascontiguousarray(np.kron(eye8, np.ones((1, C), np.float32))).astype(NPBF),
        iota_rep=np.ascontiguousarray(
            np.tile(np.tile(np.arange(C, dtype=np.float32), BL)[:, None],
                    (1, CHW))).astype(NPBF),
    )


LAST_RESULTS = {}


def _run(nc, in_maps, label):
    res = run_bass_kernel_spmd(nc, in_maps, core_ids=list(range(NCORES)),
                               trace=_TRACE)
    if res.exec_time_ns is not None:
        LAST_EXEC_NS[label] = res.exec_time_ns
    LAST_RESULTS[label] = res
    return res.results


def kernel(x, tags, embedding, fc_w, fc_b, start_transitions, end_transitions,
           transitions):
    x = np.asarray(x, np.int32)
    tags = np.asarray(tags, np.int32)
    embedding = np.asarray(embedding, np.float32)
    fc_w = np.asarray(fc_w, np.float32)
    fc_b = np.asarray(fc_b, np.float32)
    trans = np.asarray(transitions, np.float32)
    start = np.asarray(start_transitions, np.float32)
    end = np.asarray(end_transitions, np.float32)

    prep = _host_prep(embedding, fc_w, fc_b, trans, start)

    if "t2" not in _CACHE:
        nc1 = build_t2_kernel()
        nc1.finalize()
        _CACHE["t2"] = nc1
    if "main" not in _CACHE:
        nc2 = build_main_kernel()
        nc2.finalize()
        _CACHE["main"] = nc2

    # ---- launch 1: t2 = emb_pad @ fc_w (bf16 out), vocab-sharded ----
    emb_pad_T = np.zeros((E, VPAD), np.float32)
    emb_pad_T[:, :V] = embedding.T
    in1 = [{"embT_s": np.ascontiguousarray(emb_pad_T[:, k * VSH:(k + 1) * VSH]),
            "fc_w": fc_w} for k in range(NCORES)]
    res1 = _run(_CACHE["t2"], in1, "t2")
    # t2_s comes back transposed (C, VSH)
    t2_full = np.concatenate(
        [np.asarray(res1[k]["t2_s"]).T for k in range(NCORES)], axis=0)
    t2_full = np.ascontiguousarray(t2_full)          # (VPAD, C) bf16

    # ---- launch 2: main kernel, batch-sharded ----
    # permute tokens so G comes out segment-major: position c holds token
    # sigma(c) = (c % S)*L + c//S  (inverse of t -> (t%L)*S + t//L)
    sigma = (np.arange(T) % S) * L + np.arange(T) // S
    x_perm = x[:, sigma]
    tags_m = np.where(x_perm != 0, tags[:, sigma], C).astype(NPBF)
    in2 = []
    for k in range(NCORES):
        sl = slice(k * BL, (k + 1) * BL)
        xt = x_perm[sl].reshape(BL, T // 128, 128).transpose(2, 1, 0) \
                       .reshape(128, T // 128 * BL)
        in2.append({
            "x_t": np.ascontiguousarray(xt),
            "tags_f": np.ascontiguousarray(tags_m[sl]),
            "t2": t2_full,
            "blockP": prep["blockP"], "blockPT": prep["blockPT"],
            "bcast8": prep["bcast8"],
            "iota_rep": prep["iota_rep"],
        })
    res2 = _run(_CACHE["main"], in2, "main")

    # ---- host combine (float64, vectorized) ----
    lengths = (x != 0).sum(1)                        # (B,)
    start64 = start.astype(np.float64)
    end64 = end.astype(np.float64)
    fcb64 = fc_b.astype(np.float64)
    Pe = prep["P_eff"]                               # (C, C) float64
    t264 = t2_full.astype(np.float64)                # (VPAD, C)
    exp_end = np.exp(end64)

    em_total = sum(float(np.asarray(res2[k]["num_out"], np.float64).sum())
                   for k in range(NCORES))
    r = np.concatenate(
        [np.asarray(res2[k]["r_out"]).astype(np.float64).reshape(BL, C, S)
         for k in range(NCORES)], axis=0)            # (B, C, S)
    d = np.concatenate(
        [np.asarray(res2[k]["d_out"]).astype(np.float64).reshape(BL, C, S)
         for k in range(NCORES)], axis=0)            # (B, C, S)

    num = start64[tags[:, 0]] + fcb64[tags[:, 0]]
    num += end64[tags[np.arange(B), lengths - 1]]
    # transition terms (pure tags/params, no device data)
    maskf = (x[:, 1:] != 0).astype(np.float64)
    num += (prep["trans_n"][tags[:, :-1], tags[:, 1:]] * maskf).sum(axis=1)

    # exact alpha over segment 0 (tokens 0..L-1) replaces device r[:,:,0]
    # (device r0 lacks the start-transition factor)
    alpha0 = np.exp(start64[None, :] + t264[x[:, 0]] + fcb64[None, :])  # (B, C)
    for t in range(1, L):
        w = np.exp(t264[x[:, t]] + fcb64[None, :])
        alpha0 = (alpha0 @ Pe) * w        # lengths >= T//2 > L, so no masking
    r[:, :, 0] = alpha0

    # full-segment junction chain: for s in 1..sstar-1:
    #   logZ += log(r[:,:,s-1] @ (Pe @ d[:,:,s])) - log(r[:,:,s].sum())
    sstar = (lengths - 1) // L                       # (B,)
    cs = np.einsum('cd,bds->bcs', Pe, d)             # (B, C, S)
    t1 = np.einsum('bcs,bcs->bs', r[:, :, :-1], cs[:, :, 1:])   # junction at s=1..S-1
    rs = r.sum(axis=1)                               # (B, S)
    s_idx = np.arange(1, S)[None, :]                 # (1, S-1)
    jmask = s_idx <= (sstar[:, None] - 1)            # (B, S-1)
    terms = np.where(jmask, np.log(t1) - np.log(rs[:, 1:]), 0.0)
    logZ = terms.sum(axis=1)                         # (B,)

    # ragged tail: exact alpha recursion from segment sstar-1's r
    alpha = r[np.arange(B), :, sstar - 1].copy()     # (B, C)
    tail_len = lengths - sstar * L                   # in [1, L]
    for t_off in range(L):
        active = t_off < tail_len                    # (B,)
        t_idx = np.minimum(sstar * L + t_off, T - 1)
        w = np.exp(t264[x[np.arange(B), t_idx]] + fcb64[None, :])   # (B, C)
        alpha_new = (alpha @ Pe) * w
        alpha = np.where(active[:, None], alpha_new, alpha)
    logZ += np.log(alpha @ exp_end)

    total = -(num - logZ).sum() - em_total
    return np.array(total, dtype=np.float32)


# revision 55
# speedup vs baseline: 1.6014x; 1.0726x over previous
"""CRF negative-log-likelihood kernel for Trainium2 (8 NeuronCores, batch-sharded).

Algorithm:
  - Launch 1 (vocab-sharded): t2 = embedding @ fc_w in bf16. Host pre-transposes
    the embedding shard so the kernel is just convert-to-bf16 + 50 matmuls
    (lhsT = embT chunk, rhs = fc_w), no PE transposes. Output t2 is bf16
    (32B rows) to halve gather traffic.
  - Launch 2 (batch-sharded, 8 rows/core, bf16 compute): merged indirect-DMA
    gathers of t2 rows (8 calls, 4096 descriptors each), bf16 PE-block
    transposes into class-on-partition layout, numerator via one-hot matmul +
    multiply-reduce, and a segmented forward/backward scan (L=16 steps, S=256
    segments on the free dim) in linear space with the two scan chains
    interleaved so vector muls hide behind the other chain's matmuls.
  - Host (float64, vectorized): rank-1 junction chain across segments, exact
    partial segment for each row's ragged tail, final scalar assembly.
"""
import sys
sys.path.insert(0, "/opt/trn_rl_repo")
import numpy as np
import ml_dtypes
from contextlib import ExitStack

import concourse.bass as bass
import concourse.bacc as bacc_mod
import concourse.mybir as mybir
import concourse.tile as tile
from concourse.masks import make_identity
from concourse.bass_utils import run_bass_kernel_spmd

F32 = mybir.dt.float32
BF16 = mybir.dt.bfloat16
I32 = mybir.dt.int32
NPBF = ml_dtypes.bfloat16

V, E, C = 50257, 128, 16
B, T = 64, 4096
L, S = 8, 512
VPAD = 51200
VSH = VPAD // 8
BL = 8
NCHUNK = 8
CHW = T // NCHUNK
NCORES = 8

LAST_EXEC_NS = {}
_TRACE = False
_CACHE = {}


def build_t2_kernel():
    nc = bacc_mod.Bacc()
    # embT_s: host-pretransposed shard, (E, VSH), already bf16
    embT_s = nc.dram_tensor("embT_s", [E, VSH], BF16, kind="ExternalInput")
    fc_w = nc.dram_tensor("fc_w", [E, C], F32, kind="ExternalInput")
    # t2 shard TRANSPOSED: (C, VSH); host un-transposes. One stationary fcw,
    # wide matmuls, contiguous out-DMA.
    t2_s = nc.dram_tensor("t2_s", [C, VSH], BF16, kind="ExternalOutput")

    NGRP = 10                   # DMA/matmul granularity
    GW = VSH // NGRP            # 640 columns per group
    with ExitStack() as ctx:
        tc = ctx.enter_context(tile.TileContext(nc))
        singles = ctx.enter_context(tc.tile_pool(name="singles", bufs=1))
        psum = ctx.enter_context(tc.tile_pool(name="psum", bufs=4, space="PSUM"))

        fcw_f32 = singles.tile([E, C], F32)
        nc.scalar.dma_start(out=fcw_f32[:], in_=fc_w[:])
        fcw_bf = singles.tile([E, C], BF16)
        nc.vector.tensor_copy(fcw_bf[:], fcw_f32[:])

        EMBTb = singles.tile([128, VSH], BF16)
        T2T = singles.tile([C, VSH], BF16)
        MW = GW // 2            # 320 f32 fits a 2KB PSUM bank
        for g in range(NGRP):
            eng = nc.sync if g % 2 == 0 else nc.gpsimd
            eng.dma_start(out=EMBTb[:, g * GW:(g + 1) * GW],
                          in_=embT_s[:, g * GW:(g + 1) * GW])
        for m in range(2 * NGRP):
            ps2 = psum.tile([C, MW], F32, tag="p2")
            nc.tensor.matmul(ps2[:], lhsT=fcw_bf[:],
                             rhs=EMBTb[:, m * MW:(m + 1) * MW],
                             start=True, stop=True)
            if m % 2 == 0:
                nc.vector.tensor_copy(T2T[:, m * MW:(m + 1) * MW], ps2[:])
            else:
                nc.scalar.copy(T2T[:, m * MW:(m + 1) * MW], ps2[:])
            if m == NGRP - 1:
                nc.sync.dma_start(out=t2_s[:, :NGRP * MW],
                                  in_=T2T[:, :NGRP * MW])
        nc.scalar.dma_start(out=t2_s[:, NGRP * MW:], in_=T2T[:, NGRP * MW:])
    return nc


def _strided(base_ap, k, step, count):
    return bass.AP(tensor=base_ap.tensor, offset=base_ap.offset + k,
                   ap=[base_ap.ap[0], [step, count]])


def build_main_kernel():
    nc = bacc_mod.Bacc()
    x_t = nc.dram_tensor("x_t", [128, T // 128 * BL], I32, kind="ExternalInput")
    tags_f = nc.dram_tensor("tags_f", [BL, T], BF16, kind="ExternalInput")
    t2 = nc.dram_tensor("t2", [VPAD, C], BF16, kind="ExternalInput")
    blockP = nc.dram_tensor("blockP", [128, 128], BF16, kind="ExternalInput")
    blockPT = nc.dram_tensor("blockPT", [128, 128], BF16, kind="ExternalInput")
    bcast8 = nc.dram_tensor("bcast8", [BL, 128], BF16, kind="ExternalInput")
    iota_rep = nc.dram_tensor("iota_rep", [128, CHW], BF16, kind="ExternalInput")

    r_out = nc.dram_tensor("r_out", [128, S], BF16, kind="ExternalOutput")
    d_out = nc.dram_tensor("d_out", [128, S], BF16, kind="ExternalOutput")
    num_out = nc.dram_tensor("num_out", [1, 2 * NCHUNK], F32, kind="ExternalOutput")

    with ExitStack() as ctx:
        tc = ctx.enter_context(tile.TileContext(nc))
        singles = ctx.enter_context(tc.tile_pool(name="singles", bufs=1))
        big = ctx.enter_context(tc.tile_pool(name="big", bufs=1))
        scratch = ctx.enter_context(tc.tile_pool(name="scratch", bufs=3))
        psum = ctx.enter_context(tc.tile_pool(name="psum", bufs=3, space="PSUM"))
        psumT = ctx.enter_context(tc.tile_pool(name="psumT", bufs=2, space="PSUM"))
        psum2 = ctx.enter_context(tc.tile_pool(name="psum2", bufs=1, space="PSUM"))

        # input DMAs spread across queues for parallel issue
        xt_sb = singles.tile([128, T // 128 * BL], I32)
        nc.sync.dma_start(out=xt_sb[:], in_=x_t[:])
        tagsf_sb = singles.tile([BL, T], BF16)
        nc.scalar.dma_start(out=tagsf_sb[:], in_=tags_f[:])
        bcast8_sb = singles.tile([BL, 128], BF16)
        nc.scalar.dma_start(out=bcast8_sb[:], in_=bcast8[:])
        iotar_sb = singles.tile([128, CHW], BF16)
        nc.scalar.dma_start(out=iotar_sb[:], in_=iota_rep[:])
        blockP_sb = singles.tile([128, 128], BF16)
        nc.scalar.dma_start(out=blockP_sb[:], in_=blockP[:])
        blockPT_sb = singles.tile([128, 128], BF16)
        nc.scalar.dma_start(out=blockPT_sb[:], in_=blockPT[:])

        TM = big.tile([128, T], BF16)
        G = big.tile([128, T], BF16)
        EXPG = big.tile([128, T], BF16)
        W = big.tile([128, T], BF16)
        num_sb = singles.tile([1, 2 * NCHUNK], F32)

        nc.vector.memset(num_sb[:], 0.0)

        EXPGap = EXPG[:]
        Gap = G[:]

        ones_sb = singles.tile([128, 1], BF16)
        nc.vector.memset(ones_sb[:], 1.0)
        ident_bf = singles.tile([128, 128], BF16)
        make_identity(nc, ident_bf[:])
        r_sb = big.tile([128, S], BF16)
        nc.vector.memset(r_sb[:], 1.0)
        d_sb = big.tile([128, S], BF16)

        # Host permutes the token order so that G comes out SEGMENT-MAJOR:
        # G column k*S + s = token s*L + k. Chunk c of the gather therefore
        # holds exactly scan step k=c's emission block, so the forward scan
        # chases the gather; with the balanced gather order below the
        # backward scan chases from the other end.
        GORDER = [0, 7, 1, 6, 2, 5, 3, 4]

        # --- numerator W build first: only needs tags, runs during gather ---
        for c in GORDER:
            c0 = c * CHW
            psA = psum.tile([128, CHW], F32, tag="ps")
            nc.tensor.matmul(psA[:], lhsT=bcast8_sb[:],
                             rhs=tagsf_sb[:, c0:c0 + CHW], start=True, stop=True)
            nc.vector.tensor_tensor(out=W[:, c0:c0 + CHW], in0=psA[:],
                                    in1=iotar_sb[:], op=mybir.AluOpType.is_equal)

        # --- gather: one merged indirect DMA per chunk (4096 descriptors).
        # Offsets enumerate (partition, col) C-order; each offset owns 16
        # contiguous bf16 of the dest view.
        ncc = CHW // 16
        scr2s = {}
        for c in GORDER:
            c0 = c * CHW
            nc.gpsimd.indirect_dma_start(
                out=TM[:, c0:c0 + CHW],
                out_offset=None,
                in_=t2[:],
                in_offset=bass.IndirectOffsetOnAxis(
                    ap=xt_sb[:, c * ncc:(c + 1) * ncc], axis=0),
            )

        # per-chunk pipeline pieces, issued in gather-slot order below
        nbl = CHW // 128
        psN = psum2.tile([1, CHW], F32, tag="psN")

        def xbar_exp(c):
            # PE block-transposes (keeps the DMA engines free for gather
            # descriptors); copy back on DVE (2x bf16 mode)
            c0 = c * CHW
            psT = psumT.tile([128, CHW], BF16, tag="psT")
            for b in range(nbl):
                nc.tensor.transpose(psT[:, b * 128:(b + 1) * 128],
                                    TM[:, c0 + b * 128:c0 + (b + 1) * 128],
                                    ident_bf[:])
            nc.vector.tensor_copy(G[:, c0:c0 + CHW], psT[:])
            nc.scalar.activation(EXPG[:, c0:c0 + CHW], G[:, c0:c0 + CHW],
                                 mybir.ActivationFunctionType.Exp)

        ACC_ORDER = [0, 7, 1, 6, 2, 5, 3, 4]

        def numer(c):
            # em_tag contribution: sum(G * W) via DVE mul (2x bf16 mode)
            # + ones-matmul accumulation into psN
            c0 = c * CHW
            scr2 = scratch.tile([128, CHW], BF16, tag=f"scr2_{c % 3}")
            nc.vector.tensor_mul(scr2[:], G[:, c0:c0 + CHW], W[:, c0:c0 + CHW])
            i = ACC_ORDER.index(c)
            nc.tensor.matmul(psN[:], lhsT=ones_sb[:], rhs=scr2[:],
                             start=(i == 0), stop=(i == NCHUNK - 1))

        def fw(k):
            psR = psum2.tile([128, S], F32, tag="psR")
            nc.tensor.matmul(psR[:], lhsT=blockP_sb[:], rhs=r_sb[:],
                             start=True, stop=True)
            nc.vector.tensor_mul(r_sb[:], psR[:], EXPG[:, k * S:(k + 1) * S])

        def bw(k):
            psD = psum2.tile([128, S], F32, tag="psD")
            nc.tensor.matmul(psD[:], lhsT=blockPT_sb[:], rhs=d_sb[:],
                             start=True, stop=True)
            nc.vector.tensor_mul(d_sb[:], psD[:], EXPG[:, k * S:(k + 1) * S])

        def dinit():
            nc.vector.tensor_copy(d_sb[:], EXPG[:, (L - 1) * S:L * S])

        # slot-ordered issue: each op runs as its chunk lands
        xbar_exp(0); fw(0); numer(0)
        xbar_exp(7); dinit(); numer(7)
        xbar_exp(1); fw(1); numer(1)
        xbar_exp(6); bw(6); numer(6)
        xbar_exp(2); fw(2); numer(2)
        xbar_exp(5); bw(5); numer(5)
        xbar_exp(3); fw(3); numer(3)
        xbar_exp(4); fw(4); numer(4)
        bw(4)
        fw(5)
        bw(3)
        fw(6)
        bw(2)
        fw(7)
        bw(1)
        bw(0)
        nc.vector.reduce_sum(out=num_sb[:, 0:1], in_=psN[:],
                             axis=mybir.AxisListType.X)

        nc.sync.dma_start(out=r_out[:], in_=r_sb[:])
        nc.sync.dma_start(out=num_out[:], in_=num_sb[:])
        nc.scalar.dma_start(out=d_out[:], in_=d_sb[:])
    return nc


def _host_prep(embedding, fc_w, fc_b, trans, start):
    P_eff64 = np.exp(trans.astype(np.float64) + fc_b[None, :].astype(np.float64))
    trans_n = (trans + fc_b[None, :]).astype(np.float32)
    P_eff32 = P_eff64.astype(np.float32)

    eye8 = np.eye(BL, dtype=np.float32)
    return dict(
        P_eff=P_eff64,
        trans_n=trans_n.astype(np.float64),
        blockP=np.ascontiguousarray(np.kron(eye8, P_eff32)).astype(NPBF),
        blockPT=np.ascontiguousarray(np.kron(eye8, P_eff32.T.copy())).astype(NPBF),
        bcast8=np.ascontiguousarray(np.kron(eye8, np.ones((1, C), np.float32))).astype(NPBF),
        iota_rep=np.ascontiguousarray(
            np.tile(np.tile(np.arange(C, dtype=np.float32), BL)[:, None],
                    (1, CHW))).astype(NPBF),
    )


LAST_RESULTS = {}


def _run(nc, in_maps, label):
    res = run_bass_kernel_spmd(nc, in_maps, core_ids=list(range(NCORES)),
                               trace=_TRACE)
    if res.exec_time_ns is not None:
        LAST_EXEC_NS[label] = res.exec_time_ns
    LAST_RESULTS[label] = res
    return res.results


def kernel(x, tags, embedding, fc_w, fc_b, start_transitions, end_transitions,
           transitions):
    x = np.asarray(x, np.int32)
    tags = np.asarray(tags, np.int32)
    embedding = np.asarray(embedding, np.float32)
    fc_w = np.asarray(fc_w, np.float32)
    fc_b = np.asarray(fc_b, np.float32)
    trans = np.asarray(transitions, np.float32)
    start = np.asarray(start_transitions, np.float32)
    end = np.asarray(end_transitions, np.float32)

    prep = _host_prep(embedding, fc_w, fc_b, trans, start)

    if "t2" not in _CACHE:
        nc1 = build_t2_kernel()
        nc1.finalize()
        _CACHE["t2"] = nc1
    if "main" not in _CACHE:
        nc2 = build_main_kernel()
        nc2.finalize()
        _CACHE["main"] = nc2

    # ---- launch 1: t2 = emb_pad @ fc_w (bf16 out), vocab-sharded ----
    emb_pad_T = np.zeros((E, VPAD), NPBF)
    emb_pad_T[:, :V] = embedding.T.astype(NPBF)
    in1 = [{"embT_s": np.ascontiguousarray(emb_pad_T[:, k * VSH:(k + 1) * VSH]),
            "fc_w": fc_w} for k in range(NCORES)]
    res1 = _run(_CACHE["t2"], in1, "t2")
    # t2_s comes back transposed (C, VSH)
    t2_full = np.concatenate(
        [np.asarray(res1[k]["t2_s"]).T for k in range(NCORES)], axis=0)
    t2_full = np.ascontiguousarray(t2_full)          # (VPAD, C) bf16

    # ---- launch 2: main kernel, batch-sharded ----
    # permute tokens so G comes out segment-major: position c holds token
    # sigma(c) = (c % S)*L + c//S  (inverse of t -> (t%L)*S + t//L)
    sigma = (np.arange(T) % S) * L + np.arange(T) // S
    x_perm = x[:, sigma]
    tags_m = np.where(x_perm != 0, tags[:, sigma], C).astype(NPBF)
    in2 = []
    for k in range(NCORES):
        sl = slice(k * BL, (k + 1) * BL)
        xt = x_perm[sl].reshape(BL, T // 128, 128).transpose(2, 1, 0) \
                       .reshape(128, T // 128 * BL)
        in2.append({
            "x_t": np.ascontiguousarray(xt),
            "tags_f": np.ascontiguousarray(tags_m[sl]),
            "t2": t2_full,
            "blockP": prep["blockP"], "blockPT": prep["blockPT"],
            "bcast8": prep["bcast8"],
            "iota_rep": prep["iota_rep"],
        })
    res2 = _run(_CACHE["main"], in2, "main")

    # ---- host combine (float64, vectorized) ----
    lengths = (x != 0).sum(1)                        # (B,)
    start64 = start.astype(np.float64)
    end64 = end.astype(np.float64)
    fcb64 = fc_b.astype(np.float64)
    Pe = prep["P_eff"]                               # (C, C) float64
    t264 = t2_full.astype(np.float64)                # (VPAD, C)
    exp_end = np.exp(end64)

    em_total = sum(float(np.asarray(res2[k]["num_out"], np.float64).sum())
                   for k in range(NCORES))
    r = np.concatenate(
        [np.asarray(res2[k]["r_out"]).astype(np.float64).reshape(BL, C, S)
         for k in range(NCORES)], axis=0)            # (B, C, S)
    d = np.concatenate(
        [np.asarray(res2[k]["d_out"]).astype(np.float64).reshape(BL, C, S)
         for k in range(NCORES)], axis=0)            # (B, C, S)

    num = start64[tags[:, 0]] + fcb64[tags[:, 0]]
    num += end64[tags[np.arange(B), lengths - 1]]
    # transition terms (pure tags/params, no device data)
    maskf = (x[:, 1:] != 0).astype(np.float64)
    num += (prep["trans_n"][tags[:, :-1], tags[:, 1:]] * maskf).sum(axis=1)

    # exact alpha over segment 0 (tokens 0..L-1) replaces device r[:,:,0]
    # (device r0 lacks the start-transition factor)
    alpha0 = np.exp(start64[None, :] + t264[x[:, 0]] + fcb64[None, :])  # (B, C)
    for t in range(1, L):
        w = np.exp(t264[x[:, t]] + fcb64[None, :])
        alpha0 = (alpha0 @ Pe) * w        # lengths >= T//2 > L, so no masking
    r[:, :, 0] = alpha0

    # full-segment junction chain: for s in 1..sstar-1:
    #   logZ += log(r[:,:,s-1] @ (Pe @ d[:,:,s])) - log(r[:,:,s].sum())
    sstar = (lengths - 1) // L                       # (B,)
    cs = np.einsum('cd,bds->bcs', Pe, d)             # (B, C, S)
    t1 = np.einsum('bcs,bcs->bs', r[:, :, :-1], cs[:, :, 1:])   # junction at s=1..S-1
    rs = r.sum(axis=1)                               # (B, S)
    s_idx = np.arange(1, S)[None, :]                 # (1, S-1)
    jmask = s_idx <= (sstar[:, None] - 1)            # (B, S-1)
    terms = np.where(jmask, np.log(t1) - np.log(rs[:, 1:]), 0.0)
    logZ = terms.sum(axis=1)                         # (B,)

    # ragged tail: exact alpha recursion from segment sstar-1's r
    alpha = r[np.arange(B), :, sstar - 1].copy()     # (B, C)
    tail_len = lengths - sstar * L                   # in [1, L]
    for t_off in range(L):
        active = t_off < tail_len                    # (B,)
        t_idx = np.minimum(sstar * L + t_off, T - 1)
        w = np.exp(t264[x[np.arange(B), t_idx]] + fcb64[None, :])   # (B, C)
        alpha_new = (alpha @ Pe) * w
        alpha = np.where(active[:, None], alpha_new, alpha)
    logZ += np.log(alpha @ exp_end)

    total = -(num - logZ).sum() - em_total
    return np.array(total, dtype=np.float32)
